# revision 32
# baseline (speedup 1.0000x reference)
"""MoE gate (group-limited greedy routing) on 8 Trainium2 NeuronCores.

Math (per token t):
    logits = x[t, 1:] @ weight.T                    (64 experts)
    scores = sigmoid(logits)
    sb     = scores + bias
    group_scores[g] = sum(top2(sb[g*8:(g+1)*8]))    (8 groups)
    keep top-4 groups; mask the rest to -inf
    top-8 experts of masked sb -> indices
    weights = 2.5 * normalize(scores[indices])

Device strategy per core (4096 tokens), fp8res path (default):
  - host splits x[:, 1:].T (feature-major, padded to 2048 rows) into
    xh = fp16(x) and xd = e4m3((x - xh) * 2^12), both stored partition-major
    per 512-token chunk ([128, nchunk, 16*512]) so each chunk loads with ONE
    fully-contiguous DMA per dtype (16 KB / 8 KB per-partition runs,
    ~355 GB/s = peak; 24 MB/core total vs 32 MB for fp16 hi/lo).
  - matmul per chunk: 16 k-tiles of xh stream through packed [wh|wl] fp16
    stationaries -> psum[0:64] += wh.xh, psum[64:128] += wl.xh; then 16
    k-tiles of xd (fp8e4 moving x fp16 stationary wh*2^-12) accumulate into
    psum[0:64] of the SAME group. Dropped wl.xd term is O(2^-23). Total
    logits error ~6e-5 rel, ~7/32768 token index flips (near-ties).
  - evac via ACT copies, PE transpose-accumulate back to [128 tokens, 64],
    sigmoid on ACT.
  - top-k on DVE: group top-2 via reduce-max + masked reduce-max, top-8 via
    max8/max_index, ordered score gather fused as 3 wide ops on
    [128, 4, 8, 64] (eq/mult/reduce) -- beats 32 narrow accumulating STTs
    (sequencer-bound). Output DMA on the ACT HWDGE ring.
Measured 92 us/iter with unroll=4 (baseline 209/156 us); DMA floor 68 us.
"""

import sys

sys.path.insert(0, "/opt/trn_rl_repo")

import numpy as np
import concourse.bacc as bacc
import concourse.mybir as mybir
from concourse.tile import TileContext
from concourse.bass_utils import run_bass_kernel_spmd

F32 = mybir.dt.float32
F16 = mybir.dt.float16
F8E5 = mybir.dt.float8e5
F8E4 = mybir.dt.float8e4
U32 = mybir.dt.uint32
I32 = mybir.dt.int32
Alu = mybir.AluOpType
Act = mybir.ActivationFunctionType
AxX = mybir.AxisListType.X

T = 32768
DIM = 2048
E = 64
G = 8
GS = E // G          # 8 experts per group
TOPK = 8
ROUTE_SCALE = 2.5

NCORES = 8
TPC = T // NCORES    # 4096 tokens per core
CHUNK = 512          # tokens per matmul chunk
NCHUNK = TPC // CHUNK
KP = 128             # contraction tile
KT = DIM // KP       # 16 k-tiles (feature dim padded 2047 -> 2048)

NEG = -1.0e9
SD = 12              # residual pre-scale exponent (fp8 stream carries d*2^SD)

_CACHE = {}


def _topk_tile(nc, pool, sc, br_sb, negc, w_out, i_out, row0, cfg):
    """Group-limited top-8 for one [128 tokens, 64 experts] score tile.

    cfg keys select engine for elementwise work: 'ew' (nc.vector or
    nc.gpsimd), 'gather_split' = how many of the 8 gather ops go to gpsimd.
    """
    P = 128
    ew = nc.gpsimd if cfg.get("ew_gpsimd") else nc.vector

    sb = pool.tile([P, E], F32, tag="sb")
    ew.tensor_add(sb[:], sc[:], br_sb[:])
    sbg = sb[:].rearrange("p (g s) -> p g s", s=GS)

    # group top-2 sum: m1 = group max; m2 = max with m1 removed
    m1 = pool.tile([P, G], F32, tag="m1")
    nc.vector.tensor_reduce(m1[:], sbg, axis=AxX, op=Alu.max)
    eq = pool.tile([P, E], F32, tag="eqg")
    ew.tensor_tensor(
        eq[:].rearrange("p (g s) -> p g s", s=GS), sbg,
        m1[:].unsqueeze(2).to_broadcast([P, G, GS]), op=Alu.is_equal)
    sb2 = pool.tile([P, E], F32, tag="sb2")
    ew.scalar_tensor_tensor(
        out=sb2[:], in0=eq[:], scalar=NEG, in1=sb[:],
        op0=Alu.mult, op1=Alu.add)
    m2 = pool.tile([P, G], F32, tag="m2")
    nc.vector.tensor_reduce(
        m2[:], sb2[:].rearrange("p (g s) -> p g s", s=GS), axis=AxX, op=Alu.max)
    gs_t = pool.tile([P, G], F32, tag="gs")
    ew.tensor_add(gs_t[:], m1[:], m2[:])

    # threshold = 4th largest group score; penalty -1e9 for dropped groups
    g8 = pool.tile([P, 8], F32, tag="g8")
    nc.vector.max(out=g8[:], in_=gs_t[:])
    pen = pool.tile([P, G], F32, tag="pen")
    ew.scalar_tensor_tensor(
        out=pen[:], in0=gs_t[:], scalar=g8[:, 3:4], in1=negc[:],
        op0=Alu.is_lt, op1=Alu.mult)

    mk = pool.tile([P, E], F32, tag="mk")
    ew.tensor_tensor(
        mk[:].rearrange("p (g s) -> p g s", s=GS), sbg,
        pen[:].unsqueeze(2).to_broadcast([P, G, GS]), op=Alu.add)

    # top-8 experts of masked sb (values sorted desc + their indices)
    v8 = pool.tile([P, 8], F32, tag="v8")
    nc.vector.max(out=v8[:], in_=mk[:])
    ix = pool.tile([P, 8], U32, tag="ix")
    nc.vector.max_index(out=ix[:], in_max=v8[:], in_values=mk[:])

    # ordered gather of original scores: (mk == v8[j]) * scores, summed
    gat = pool.tile([P, 8], F32, tag="gat")
    junk = pool.tile([P, E], F32, tag="junk")
    junk2 = pool.tile([P, E], F32, tag="junk2")
    n_gp = cfg.get("gather_gpsimd", 0)
    for j in range(TOPK):
        eng = nc.gpsimd if j < n_gp else nc.vector
        eng.scalar_tensor_tensor(
            out=(junk2 if j < n_gp else junk)[:],
            in0=mk[:], scalar=v8[:, j:j + 1], in1=sc[:],
            op0=Alu.is_equal, op1=Alu.mult, accum_out=gat[:, j:j + 1])

    # normalize * 2.5
    s1 = pool.tile([P, 1], F32, tag="s1")
    nc.vector.tensor_reduce(s1[:], gat[:], axis=AxX, op=Alu.add)
    r1 = pool.tile([P, 1], F32, tag="r1")
    nc.vector.reciprocal(r1[:], s1[:])
    wo = pool.tile([P, 8], F32, tag="wo")
    ew.tensor_scalar(
        out=wo[:], in0=gat[:], scalar1=r1[:, 0:1], scalar2=float(ROUTE_SCALE),
        op0=Alu.mult, op1=Alu.mult)

    nc.sync.dma_start(w_out[row0:row0 + P, :], wo[:])
    nc.sync.dma_start(i_out[row0:row0 + P, :], ix[:].bitcast(I32))


def _body(nc, pools, dram, cfg):
    cpool, xpool, wpool, psA, psB = pools
    xt, w_out, i_out, wt_sb, br_sb, id_sb, negc, br4 = dram
    mode = cfg.get("mode", "full")

    f32mm = cfg.get("f32mm")
    fp8res = cfg.get("fp8res", False)
    CH0 = cfg.get("chunk", CHUNK)
    if cfg.get("ramp") and not fp8res:
        sched = [(0, 256), (256, 256)]
        t = 512
        while t < TPC:
            sched.append((t, CH0))
            t += CH0
    else:
        sched = [(c * CH0, CH0) for c in range(TPC // CH0)]

    if fp8res:
        # xt here is (xh_dram, xd_dram, wd_sb): partition-major token chunks
        xh_dram, xd_dram, wd_sb = xt
        nsp = cfg.get("dma_split", 1)
        for ci, (t0, CH) in enumerate(sched):
            xh_t = xpool.tile([KP, KT * CH], F16, tag="xh")
            xd_t = xpool.tile([KP, KT * CH], F8E4, tag="xd")
            if nsp == 1:
                nc.sync.dma_start(xh_t[:], xh_dram[:, ci, :])
                xde = nc.scalar if cfg.get("xd_act") else nc.sync
                xde.dma_start(xd_t[:], xd_dram[:, ci, :])
            else:
                step = (KT * CH) // nsp
                for q in range(nsp):
                    qs = slice(q * step, (q + 1) * step)
                    nc.sync.dma_start(xh_t[:, qs], xh_dram[:, ci, qs])
                    nc.sync.dma_start(xd_t[:, qs], xd_dram[:, ci, qs])

            if mode == "dma":
                zz = wpool.tile([KP, 1], F32, tag="zz")
                nc.vector.tensor_reduce(zz[:], xh_t[:, 0:8], axis=AxX,
                                        op=Alu.max)
                zz2 = wpool.tile([KP, 1], F32, tag="zz2")
                nc.vector.tensor_reduce(zz2[:], xd_t[:, 0:8].bitcast(mybir.dt.uint8),
                                        axis=AxX, op=Alu.max)
                continue

            xhv = xh_t[:].rearrange("p (k c) -> p k c", k=KT)
            xdv = xd_t[:].rearrange("p (k c) -> p k c", k=KT)
            ps = psA.tile([2 * E, CH], F32, tag="mm")
            NH = max(1, CH // 512)
            for k in range(KT):
                for h in range(NH):
                    hs = slice(h * 512, min((h + 1) * 512, CH))
                    nc.tensor.matmul(ps[:, hs],
                                     wt_sb[:, k * 2 * E:(k + 1) * 2 * E],
                                     xhv[:, k, hs], start=(k == 0 and h == 0),
                                     stop=False)
            for k in range(KT):
                for h in range(NH):
                    hs = slice(h * 512, min((h + 1) * 512, CH))
                    nc.tensor.matmul(ps[0:E, hs], wd_sb[:, k * E:(k + 1) * E],
                                     xdv[:, k, hs], start=False,
                                     stop=(k == KT - 1 and h == NH - 1),
                                     skip_group_check=True)

            ec = nc.vector.tensor_copy if cfg.get("evac") == "dve" \
                else nc.scalar.copy
            lg = wpool.tile([E, CH], F32, tag="lg")
            ec(lg[:], ps[0:E, :])
            lgB = wpool.tile([E, CH], F32, tag="lgB")
            ec(lgB[:], ps[E:2 * E, :])

            NB = CH // 128
            pt = psB.tile([128, NB, E], F32, tag="pt")
            for j in range(NB):
                js = slice(j * 128, (j + 1) * 128)
                nc.tensor.matmul(pt[:, j, :], lg[:, js], id_sb[:],
                                 is_transpose=True, start=True, stop=False)
                nc.tensor.matmul(pt[:, j, :], lgB[:, js], id_sb[:],
                                 is_transpose=True, start=False, stop=True)
            if cfg.get("topk_pair") and mode not in ("mm",):
                # batch two chunks' topk into one NB=2*NB pass (fewer,
                # wider DVE ops)
                if ci % 2 == 0:
                    scW = wpool.tile([128, 2 * NB, E], F32, tag="scW")
                    pair_t0 = t0
                half = ci % 2
                nc.scalar.activation(scW[:, half * NB:(half + 1) * NB, :],
                                     pt[:], Act.Sigmoid)
                if half == 1:
                    _topk_chunk(nc, wpool, scW, br4, w_out, i_out, pair_t0,
                                cfg, 2 * CH)
                continue
            sc = wpool.tile([128, NB, E], F32, tag="sc")
            nc.scalar.activation(sc[:], pt[:], Act.Sigmoid)
            if mode == "mm":
                nc.sync.dma_start(w_out[t0:t0 + 128, :], sc[:, 0, 0:8])
                continue
            _topk_chunk(nc, wpool, sc, br4, w_out, i_out, t0, cfg, CH)
        return

    for t0, CH in sched:
        if f32mm:
            xk = []
            for k in range(KT):
                tl = xpool.tile([KP, CH], F32, tag="xt")
                nc.sync.dma_start(tl[:],
                                  xt[k * KP:(k + 1) * KP, t0:t0 + CH])
                xk.append(tl)
        else:
            xk = []
            for k in range(KT):
                th = xpool.tile([KP, 2, CH], F16, tag="xhl")
                nc.sync.dma_start(th[:],
                                  xt[k * KP:(k + 1) * KP, :, t0:t0 + CH])
                xk.append(th)

        if mode == "dma":
            # consume tiles with a trivial reduce so loads are not dead
            zz = wpool.tile([KP, 1], F32, tag="zz")
            nc.vector.tensor_reduce(zz[:], xk[0][:, 0, 0:8] if not f32mm
                                    else xk[0][:, 0:8], axis=AxX, op=Alu.max)
            continue

        if f32mm:
            ps = psA.tile([E, CH], F32, tag="mm")
            for k in range(KT):
                nc.tensor.matmul(
                    ps[:], wt_sb[:, k * E:(k + 1) * E], xk[k][:],
                    start=(k == 0), stop=(k == KT - 1))
            lg = wpool.tile([E, CH], F32, tag="lg")
            if cfg.get("evac", "act") == "act":
                nc.scalar.copy(lg[:], ps[:])
            else:
                nc.vector.tensor_copy(lg[:], ps[:])
        else:
            # fp32 via fp16 hi/lo split with a packed [wh|wl] stationary:
            # streaming xh then xl through the 128-wide array accumulates
            # psum[0:64]  = wh.xh + wh.xl
            # psum[64:128]= wl.xh + wl.xl
            # so lower+upper = (wh+wl).(xh+xl) = the full-precision product.
            # 2 streams per k-tile instead of 4 (native fp32) or 3 (naive).
            ps = psA.tile([2 * E, CH], F32, tag="mm")
            NH = max(1, CH // 512)
            n = 0
            for k in range(KT):
                wk = wt_sb[:, k * 2 * E:(k + 1) * 2 * E]
                for half in (0, 1):
                    for h in range(NH):
                        hs = slice(h * 512, min((h + 1) * 512, CH))
                        nc.tensor.matmul(ps[:, hs], wk, xk[k][:, half, hs],
                                         start=(n == 0),
                                         stop=(n >= 2 * KT - 1))
                    n += 1
            if cfg.get("acc_tp", True):
                ec = nc.vector.tensor_copy if cfg.get("evac") == "dve" \
                    else nc.scalar.copy
                lg = wpool.tile([E, CH], F32, tag="lg")
                ec(lg[:], ps[0:E, :])
                lgB = wpool.tile([E, CH], F32, tag="lgB")
                ec(lgB[:], ps[E:2 * E, :])
            else:
                tmp = wpool.tile([E, CH], F32, tag="tmphalf")
                nc.scalar.copy(tmp[:], ps[E:2 * E, :])
                lg = wpool.tile([E, CH], F32, tag="lg")
                nc.vector.tensor_add(lg[:], ps[0:E, :], tmp[:])

        if cfg.get("per_tile"):
            for j in range(CH // 128):
                pt = psB.tile([128, E], F32, tag="pt")
                nc.tensor.transpose(pt[:], lg[:, j * 128:(j + 1) * 128],
                                    id_sb[:])
                sc = wpool.tile([128, E], F32, tag="sc")
                nc.scalar.activation(sc[:], pt[:], Act.Sigmoid)
                if mode == "mm":
                    nc.sync.dma_start(
                        w_out[t0 + j * 128:t0 + (j + 1) * 128, :], sc[:, 0:8])
                    continue
                _topk_tile(nc, wpool, sc, br_sb, negc,
                           w_out, i_out, t0 + j * 128, cfg)
            continue

        # blocked layout: token-tiles side by side on the free dim so the
        # elementwise work runs as one wide op per step
        NB = CH // 128
        pt = psB.tile([128, NB, E], F32, tag="pt")
        if not f32mm and cfg.get("acc_tp", True):
            # lg holds [wh-half; wl-half]: accumulate both transposes into
            # the same psum tile -> logits.T without a separate halves-add
            for j in range(NB):
                js = slice(j * 128, (j + 1) * 128)
                nc.tensor.matmul(pt[:, j, :], lg[:, js], id_sb[:],
                                 is_transpose=True, start=True, stop=False)
                nc.tensor.matmul(pt[:, j, :], lgB[:, js], id_sb[:],
                                 is_transpose=True, start=False, stop=True)
        else:
            for j in range(NB):
                nc.tensor.transpose(pt[:, j, :], lg[:, j * 128:(j + 1) * 128],
                                    id_sb[:])
        sc = wpool.tile([128, NB, E], F32, tag="sc")
        nc.scalar.activation(sc[:], pt[:], Act.Sigmoid)
        if mode == "mm":
            nc.sync.dma_start(w_out[t0:t0 + 128, :], sc[:, 0, 0:8])
            continue
        _topk_chunk(nc, wpool, sc, br4, w_out, i_out, t0, cfg, CH)


def _topk_chunk(nc, pool, sc, br4, w_out, i_out, t0, cfg, CH=None):
    """Group-limited top-8 for one [128, NB, 64] blocked score chunk."""
    P = 128
    NB = (CH or cfg.get("chunk", CHUNK)) // 128
    V = nc.vector

    def t4(ap):  # [P, NB, G, GS] view
        return ap.rearrange("p b (g s) -> p b g s", s=GS)

    sb = pool.tile([P, NB, E], F32, tag="sb")
    eng_sb = nc.gpsimd if cfg.get("gp_adds") else V
    eng_sb.tensor_add(sb[:], sc[:], br4[:, 0:NB, :])

    # group top-2 sum: m1 = group max, m2 = max after masking m1 out
    m1 = pool.tile([P, NB, G], F32, tag="m1")
    V.tensor_reduce(m1[:], t4(sb[:]), axis=AxX, op=Alu.max)
    eq = pool.tile([P, NB, E], F32, tag="eqg")
    V.tensor_tensor(t4(eq[:]), t4(sb[:]),
                    m1[:].unsqueeze(3).to_broadcast([P, NB, G, GS]),
                    op=Alu.is_equal)
    sb2 = pool.tile([P, NB, E], F32, tag="sb2")
    V.scalar_tensor_tensor(out=sb2[:], in0=eq[:], scalar=NEG, in1=sb[:],
                           op0=Alu.mult, op1=Alu.add)
    m2 = pool.tile([P, NB, G], F32, tag="m2")
    V.tensor_reduce(m2[:], t4(sb2[:]), axis=AxX, op=Alu.max)
    gs_t = pool.tile([P, NB, G], F32, tag="gs")
    eng_sb.tensor_add(gs_t[:], m1[:], m2[:])

    # per-token group keep-mask: drop groups with rank >= TOPK_G
    if cfg.get("cnt_top4", False):
        # rank[g] = #{g': gs[g'] > gs[g]} via one wide compare + reduce;
        # replaces 4 Max8 calls (high per-op sequencer cost)
        cmp = pool.tile([P, NB, G, G], F32, tag="cmpg")
        V.tensor_tensor(cmp[:],
                        gs_t[:].unsqueeze(2).to_broadcast([P, NB, G, G]),
                        gs_t[:].unsqueeze(3).to_broadcast([P, NB, G, G]),
                        op=Alu.is_gt)
        rank = pool.tile([P, NB, G], F32, tag="rankg")
        V.tensor_reduce(rank[:], cmp[:], axis=AxX, op=Alu.add)
        pen = pool.tile([P, NB, G], F32, tag="pen")
        V.tensor_scalar(out=pen[:], in0=rank[:], scalar1=3.5, scalar2=1.0,
                        op0=Alu.is_gt, op1=Alu.mult)
    else:
        g8 = pool.tile([P, NB, 8], F32, tag="g8")
        for b in range(NB):
            V.max(out=g8[:, b, :], in_=gs_t[:, b, :])
        pen = pool.tile([P, NB, G], F32, tag="pen")
        V.tensor_tensor(pen[:], gs_t[:],
                        g8[:, :, 3:4].to_broadcast([P, NB, G]), op=Alu.is_lt)
    mk = pool.tile([P, NB, E], F32, tag="mk")
    V.scalar_tensor_tensor(
        out=t4(mk[:]),
        in0=pen[:].unsqueeze(3).to_broadcast([P, NB, G, GS]),
        scalar=NEG, in1=t4(sb[:]), op0=Alu.mult, op1=Alu.add)

    # per-token top-8 (sorted values + indices)
    v8 = pool.tile([P, NB, 8], F32, tag="v8")
    ix = pool.tile([P, NB, 8], U32, tag="ix")
    for b in range(NB):
        V.max(out=v8[:, b, :], in_=mk[:, b, :])
        V.max_index(out=ix[:, b, :], in_max=v8[:, b, :], in_values=mk[:, b, :])

    if cfg.get("mode") == "topk1":
        wv = w_out[t0:t0 + NB * 128, :].rearrange("(b p) j -> p b j", p=128)
        iv = i_out[t0:t0 + NB * 128, :].rearrange("(b p) j -> p b j", p=128)
        oeng = nc.scalar if cfg.get("out_dma_act", True) else nc.sync
        oeng.dma_start(wv, v8[:])
        oeng.dma_start(iv, ix[:].bitcast(I32))
        return

    # ordered gather of original scores: (mk == v8[j]) * scores, summed over
    # experts -- fused as 3 wide ops on [P, NB, 8, E] instead of 32 narrow
    # accumulating ops (DVE sequencer-bound otherwise)
    gat = pool.tile([P, NB, 8], F32, tag="gat")
    if cfg.get("gather_split"):
        # split the 8 per-block gather ops between GPSIMD and DVE
        n_gp = cfg["gather_split"]
        junk = pool.tile([P, E], F32, tag="junk")
        junk2 = pool.tile([P, E], F32, tag="junk2")
        for b in range(NB):
            for j in range(TOPK):
                eng = nc.gpsimd if j < n_gp else V
                eng.scalar_tensor_tensor(
                    out=(junk2 if j < n_gp else junk)[:],
                    in0=mk[:, b, :], scalar=v8[:, b, j:j + 1],
                    in1=sc[:, b, :], op0=Alu.is_equal, op1=Alu.mult,
                    accum_out=gat[:, b, j:j + 1])
    elif cfg.get("fused_gather", True):
        GE = nc.gpsimd if cfg.get("gather_gp") else V
        eq4 = pool.tile([P, NB, 8, E], F32, tag="eq4",
                        bufs=cfg.get("gbufs", 2))
        GE.tensor_tensor(
            eq4[:], mk[:].unsqueeze(2).to_broadcast([P, NB, 8, E]),
            v8[:].unsqueeze(3).to_broadcast([P, NB, 8, E]), op=Alu.is_equal)
        prod = pool.tile([P, NB, 8, E],
                         mybir.dt.bfloat16 if cfg.get("prod_bf16") else F32,
                         tag="prod", bufs=cfg.get("gbufs", 2))
        GE.tensor_tensor(
            prod[:], eq4[:], sc[:].unsqueeze(2).to_broadcast([P, NB, 8, E]),
            op=Alu.mult)
        V.tensor_reduce(gat[:], prod[:], axis=AxX, op=Alu.add)
    else:
        junk = pool.tile([P, E], F32, tag="junk")
        for b in range(NB):
            for j in range(TOPK):
                V.scalar_tensor_tensor(
                    out=junk[:], in0=mk[:, b, :], scalar=v8[:, b, j:j + 1],
                    in1=sc[:, b, :], op0=Alu.is_equal, op1=Alu.mult,
                    accum_out=gat[:, b, j:j + 1])

    # weights = 2.5 * gat / sum(gat)
    s1 = pool.tile([P, NB], F32, tag="s1")
    V.tensor_reduce(s1[:], gat[:], axis=AxX, op=Alu.add)
    r1 = pool.tile([P, NB], F32, tag="r1")
    V.reciprocal(r1[:], s1[:])
    wo = pool.tile([P, NB, 8], F32, tag="wo")
    V.scalar_tensor_tensor(
        out=wo[:], in0=gat[:], scalar=float(ROUTE_SCALE),
        in1=r1[:].unsqueeze(2).to_broadcast([P, NB, 8]),
        op0=Alu.mult, op1=Alu.mult)

    wv = w_out[t0:t0 + NB * 128, :].rearrange("(b p) j -> p b j", p=128)
    iv = i_out[t0:t0 + NB * 128, :].rearrange("(b p) j -> p b j", p=128)
    oeng = nc.scalar if cfg.get("out_dma_act", True) else nc.sync
    oeng.dma_start(wv, wo[:])
    oeng.dma_start(iv, ix[:].bitcast(I32))


def _build_nc(n_repeat=1, **cfg):
    import contextlib
    nc = bacc.Bacc(None, target_bir_lowering=False, debug=False)

    fp8res = cfg.get("fp8res", False)
    CH0 = cfg.get("chunk", CHUNK)
    if fp8res:
        nch = TPC // CH0
        xh_d = nc.declare_dram_parameter("xh", [KP, nch, KT * CH0], F16,
                                         isOutput=False)
        xd_d = nc.declare_dram_parameter("xd", [KP, nch, KT * CH0], F8E4,
                                         isOutput=False)
        wt = nc.declare_dram_parameter("whl", [KT * KP, 2 * E], F16,
                                       isOutput=False)
        wd = nc.declare_dram_parameter("wd", [KT * KP, E], F16,
                                       isOutput=False)
        xt = None
    elif cfg.get("f32mm"):
        xt = nc.declare_dram_parameter("xt", [KT * KP, TPC], F32,
                                       isOutput=False)
        wt = nc.declare_dram_parameter("wt", [KT * KP, E], F32, isOutput=False)
    else:
        xt = nc.declare_dram_parameter("xhl", [KT * KP, 2, TPC], F16,
                                       isOutput=False)
        wt = nc.declare_dram_parameter("whl", [KT * KP, 2 * E], F16,
                                       isOutput=False)
    br = nc.declare_dram_parameter("br", [128, E], F32, isOutput=False)
    idn = nc.declare_dram_parameter("idn", [E, E], F32, isOutput=False)
    w_out = nc.declare_dram_parameter("w_out", [TPC, TOPK], F32, isOutput=True)
    i_out = nc.declare_dram_parameter("i_out", [TPC, TOPK], I32, isOutput=True)

    with TileContext(nc) as tc:
        with (
            tc.tile_pool(name="const", bufs=1) as cpool,
            tc.tile_pool(name="xts",
                         bufs=cfg.get("xbufs", 4 if fp8res else 36)) as xpool,
            tc.tile_pool(name="work", bufs=cfg.get("wbufs", 4)) as wpool,
            tc.tile_pool(name="psmm", bufs=cfg.get("psa", 2),
                         space="PSUM") as psA,
            tc.tile_pool(name="pstr", bufs=cfg.get("psb", 4),
                         space="PSUM") as psB,
        ):
            if cfg.get("f32mm"):
                wt_sb = cpool.tile([KP, KT * E], F32)
                nc.sync.dma_start(
                    wt_sb[:].rearrange("p (k e) -> p k e", k=KT),
                    wt[:, :].rearrange("(k p) e -> p k e", p=KP))
            else:
                wt_sb = cpool.tile([KP, KT * 2 * E], F16)
                nc.sync.dma_start(
                    wt_sb[:].rearrange("p (k e) -> p k e", k=KT),
                    wt[:, :].rearrange("(k p) e -> p k e", p=KP))
            if fp8res:
                wd_sb = cpool.tile([KP, KT * E], F16)
                nc.sync.dma_start(
                    wd_sb[:].rearrange("p (k e) -> p k e", k=KT),
                    wd[:, :].rearrange("(k p) e -> p k e", p=KP))
                xt = (xh_d, xd_d, wd_sb)
            br_sb = cpool.tile([128, E], F32)
            nc.sync.dma_start(br_sb[:], br[:, :])
            id_sb = cpool.tile([E, E], F32)
            nc.sync.dma_start(id_sb[:], idn[:, :])
            negc = cpool.tile([128, G], F32)
            nc.vector.memset(negc[:], NEG)
            nbr = (CH0 // 128) * (2 if cfg.get("topk_pair") else 1)
            br4 = cpool.tile([128, nbr, E], F32)
            for b in range(nbr):
                nc.vector.tensor_copy(br4[:, b, :], br_sb[:])

            pools = (cpool, xpool, wpool, psA, psB)
            dram = (xt, w_out, i_out, wt_sb, br_sb, id_sb, negc, br4)
            rep_ctx = tc.For_i(0, n_repeat, 1) if n_repeat > 1 \
                else contextlib.nullcontext()
            with rep_ctx:
                for _ in range(cfg.get("unroll", 1)):
                    _body(nc, pools, dram, cfg)

    nc.compile()
    return nc


def _get_nc():
    if "nc" not in _CACHE:
        _CACHE["nc"] = _build_nc(fp8res=True)
    return _CACHE["nc"]


def _prep_inputs(x, weight, bias, f32mm=False, fp8res=True, chunk=CHUNK):
    import ml_dtypes

    x = np.asarray(x, dtype=np.float32)
    weight = np.asarray(weight, dtype=np.float32)
    bias = np.asarray(bias, dtype=np.float32)
    assert x.shape == (T, DIM) and weight.shape == (E, DIM - 1)

    br = np.tile(bias[None, :], (128, 1)).astype(np.float32)
    idn = np.eye(E, dtype=np.float32)

    wt = np.zeros((KT * KP, E), dtype=np.float32)
    wt[:DIM - 1] = weight.T
    in_maps = []
    if f32mm:
        for c in range(NCORES):
            xtc = np.zeros((KT * KP, TPC), dtype=np.float32)
            xtc[:DIM - 1] = x[c * TPC:(c + 1) * TPC, 1:].T
            in_maps.append({"xt": xtc, "wt": wt, "br": br, "idn": idn})
        return in_maps

    whl = np.empty((KT * KP, 2 * E), dtype=np.float16)
    whl[:, :E] = wt
    whl[:, E:] = wt - whl[:, :E].astype(np.float32)

    if fp8res:
        # fp16 hi + scaled-e4m3 residual, partition-major chunk layout:
        # dram[p, ci, k*CH + c] = feature (k*128+p) of token (ci*CH + c)
        nch = TPC // chunk
        wd = (whl[:, :E].astype(np.float32) * 2.0 ** -SD).astype(np.float16)
        for c in range(NCORES):
            xtc = np.zeros((KT * KP, TPC), dtype=np.float32)
            xtc[:DIM - 1] = x[c * TPC:(c + 1) * TPC, 1:].T
            xh = xtc.astype(np.float16)
            xd32 = (xtc - xh.astype(np.float32)) * float(2.0 ** SD)
            xd = xd32.astype(ml_dtypes.float8_e4m3)
            # [KT*KP, TPC] -> [KT, KP, nch, CH] -> [KP, nch, KT, CH]
            xh_p = np.ascontiguousarray(
                xh.reshape(KT, KP, nch, chunk).transpose(1, 2, 0, 3)
            ).reshape(KP, nch, KT * chunk)
            xd_p = np.ascontiguousarray(
                xd.reshape(KT, KP, nch, chunk).transpose(1, 2, 0, 3)
            ).reshape(KP, nch, KT * chunk)
            in_maps.append({"xh": xh_p, "xd": xd_p, "whl": whl, "wd": wd,
                            "br": br, "idn": idn})
        return in_maps

    for c in range(NCORES):
        xtc = np.zeros((KT * KP, TPC), dtype=np.float32)
        xtc[:DIM - 1] = x[c * TPC:(c + 1) * TPC, 1:].T
        xhl = np.empty((KT * KP, 2, TPC), dtype=np.float16)
        xhl[:, 0, :] = xtc
        xhl[:, 1, :] = xtc - xhl[:, 0, :].astype(np.float32)
        in_maps.append({"xhl": xhl, "whl": whl, "br": br, "idn": idn})
    return in_maps


def kernel(x, weight, bias):
    nc = _get_nc()
    in_maps = _prep_inputs(x, weight, bias)
    out = run_bass_kernel_spmd(nc, in_maps, list(range(NCORES)))
    _CACHE["last_result"] = out
    res = out.results
    weights = np.concatenate([res[c]["w_out"] for c in range(NCORES)], axis=0)
    indices = np.concatenate([res[c]["i_out"] for c in range(NCORES)], axis=0)
    return weights, indices


# ---------------------------------------------------------------------------
# benchmarking helpers (not used by the grader; kernel() above is the entry)
# ---------------------------------------------------------------------------

def _timed_runner(nc, in_maps):
    """Mirror bass2jax.run_bass_via_pjrt's multi-core path, but keep inputs
    resident on device and return a closure that runs + blocks."""
    import jax
    from jax.sharding import Mesh, PartitionSpec, NamedSharding
    from jax.experimental.shard_map import shard_map
    from concourse import bass2jax

    bass2jax.install_neuronx_cc_hook()
    if nc.dbg_addr is not None:
        in_maps = [
            {**m, nc.dbg_addr.name: np.zeros((1, 2), np.uint32)} for m in in_maps
        ]
    partition_name = nc.partition_id_tensor.name if nc.partition_id_tensor else None
    in_names, out_names, out_avals, zero_outs = [], [], [], []
    for alloc in nc.m.functions[0].allocations:
        if not isinstance(alloc, mybir.MemoryLocationSet):
            continue
        name = alloc.memorylocations[0].name
        if alloc.kind == "ExternalInput":
            if name != partition_name:
                in_names.append(name)
        elif alloc.kind == "ExternalOutput":
            shape = tuple(alloc.tensor_shape)
            dtype = mybir.dt.np(alloc.dtype)
            out_names.append(name)
            out_avals.append(jax.core.ShapedArray(shape, dtype))
            zero_outs.append(np.zeros(shape, dtype))
    n_params = len(in_names)
    n_cores = len(in_maps)
    all_in_names = list(in_names) + list(out_names)
    if partition_name is not None:
        all_in_names.append(partition_name)

    def _b(*args):
        operands = list(args)
        if partition_name is not None:
            operands.append(bass2jax.partition_id_tensor())
        outs = bass2jax._bass_exec_p.bind(
            *operands,
            out_avals=tuple(out_avals),
            in_names=tuple(all_in_names),
            out_names=tuple(out_names),
            lowering_input_output_aliases=(),
            sim_require_finite=True,
            sim_require_nnan=True,
            nc=nc,
        )
        return tuple(outs)

    devices = jax.devices()[:n_cores]
    mesh = Mesh(np.asarray(devices), ("core",))
    in_specs = (PartitionSpec("core"),) * (n_params + len(out_names))
    out_specs = (PartitionSpec("core"),) * len(out_names)
    fn = jax.jit(shard_map(_b, mesh=mesh, in_specs=in_specs,
                           out_specs=out_specs, check_rep=False))
    sh = NamedSharding(mesh, PartitionSpec("core"))
    concat_in = [
        jax.device_put(
            np.concatenate([np.asarray(in_maps[c][nm]) for c in range(n_cores)], 0),
            sh)
        for nm in in_names
    ]
    concat_zeros = [
        jax.device_put(np.zeros((n_cores * z.shape[0], *z.shape[1:]), z.dtype), sh)
        for z in zero_outs
    ]

    def run():
        outs = fn(*concat_in, *concat_zeros)
        jax.block_until_ready(outs)
        return outs

    return run


def bench_nc(nc_r, nc_1, in_maps, n_repeat, trials=16):
    import time
    run_r = _timed_runner(nc_r, in_maps)
    run_1 = _timed_runner(nc_1, in_maps)
    run_r(); run_1()
    ts_r, ts_1, deltas = [], [], []
    for _ in range(trials):
        t0 = time.perf_counter(); run_1(); t1 = time.perf_counter()
        run_r(); t2 = time.perf_counter()
        ts_1.append(t1 - t0); ts_r.append(t2 - t1)
        deltas.append((t2 - t1) - (t1 - t0))
    for tag, ts in ((n_repeat, ts_r), (1, ts_1)):
        print(f"    repeat={tag:3d}: min {min(ts)*1e3:8.3f} ms  "
              f"med {sorted(ts)[len(ts)//2]*1e3:8.3f} ms")
    dmin = min(ts_r) - min(ts_1)
    dmed = sorted(deltas)[len(deltas)//2]
    print(f"    delta: min-based {dmin*1e3:7.3f} ms   "
          f"median-paired {dmed*1e3:7.3f} ms")
    return min(dmin, dmed) / (n_repeat - 1) * 1e9  # per-iteration


def bench(x, weight, bias, n_repeat=256, trials=16, **cfg):
    if cfg.get("f32mm") or cfg.get("legacy"):
        cfg["fp8res"] = False
    cfg.setdefault("fp8res", True)
    if cfg["fp8res"]:
        cfg.setdefault("unroll", 4)
    u = cfg.get("unroll", 1)
    n_repeat = n_repeat // u
    in_maps = _prep_inputs(x, weight, bias, f32mm=cfg.get("f32mm", False),
                           fp8res=cfg["fp8res"],
                           chunk=cfg.get("chunk", CHUNK))
    key = tuple(sorted(cfg.items()))
    if ("ncr", key) not in _CACHE:
        _CACHE[("ncr", key)] = _build_nc(n_repeat, **cfg)
        _CACHE[("nc1", key)] = _build_nc(1, **cfg)
    per_iter = bench_nc(_CACHE[("ncr", key)], _CACHE[("nc1", key)],
                        in_maps, n_repeat, trials)
    return per_iter / u



# revision 40
# speedup vs baseline: 1.0247x; 1.0247x over previous
"""MoE gate (group-limited greedy routing) on 8 Trainium2 NeuronCores.

Math (per token t):
    logits = x[t, 1:] @ weight.T                    (64 experts)
    scores = sigmoid(logits)
    sb     = scores + bias
    group_scores[g] = sum(top2(sb[g*8:(g+1)*8]))    (8 groups)
    keep top-4 groups; mask the rest to -inf
    top-8 experts of masked sb -> indices
    weights = 2.5 * normalize(scores[indices])

Device strategy per core (4096 tokens), fp8res path (default):
  - host splits x[:, 1:].T (feature-major, padded to 2048 rows) into
    xh = fp16(x) and xd = e4m3((x - xh) * 2^12), both stored partition-major
    per 512-token chunk ([128, nchunk, 16*512]) so each chunk loads with ONE
    fully-contiguous DMA per dtype (16 KB / 8 KB per-partition runs,
    ~355 GB/s = peak; 24 MB/core total vs 32 MB for fp16 hi/lo).
  - matmul per chunk: 16 k-tiles of xh stream through packed [wh|wl] fp16
    stationaries -> psum[0:64] += wh.xh, psum[64:128] += wl.xh; then 16
    k-tiles of xd (fp8e4 moving x fp16 stationary wh*2^-12) accumulate into
    psum[0:64] of the SAME group. Dropped wl.xd term is O(2^-23). Total
    logits error ~6e-5 rel, ~7/32768 token index flips (near-ties).
  - evac via ACT copies, PE transpose-accumulate back to [128 tokens, 64],
    sigmoid on ACT.
  - top-k on DVE: group top-2 via reduce-max + masked reduce-max, top-8 via
    max8/max_index, ordered score gather fused as 3 wide ops on
    [128, 4, 8, 64] (eq/mult/reduce) -- beats 32 narrow accumulating STTs
    (sequencer-bound). Output DMA on the ACT HWDGE ring.
Measured 92 us/iter with unroll=4 (baseline 209/156 us); DMA floor 68 us.
"""

import sys

sys.path.insert(0, "/opt/trn_rl_repo")

import numpy as np
import concourse.bacc as bacc
import concourse.mybir as mybir
from concourse.tile import TileContext
from concourse.bass_utils import run_bass_kernel_spmd

F32 = mybir.dt.float32
F16 = mybir.dt.float16
F8E5 = mybir.dt.float8e5
F8E4 = mybir.dt.float8e4
U32 = mybir.dt.uint32
I32 = mybir.dt.int32
Alu = mybir.AluOpType
Act = mybir.ActivationFunctionType
AxX = mybir.AxisListType.X

T = 32768
DIM = 2048
E = 64
G = 8
GS = E // G          # 8 experts per group
TOPK = 8
ROUTE_SCALE = 2.5

NCORES = 8
TPC = T // NCORES    # 4096 tokens per core
CHUNK = 512          # tokens per matmul chunk
NCHUNK = TPC // CHUNK
KP = 128             # contraction tile
KT = DIM // KP       # 16 k-tiles (feature dim padded 2047 -> 2048)

NEG = -1.0e9
SD = 12              # residual pre-scale exponent (fp8 stream carries d*2^SD)

_CACHE = {}


def _topk_tile(nc, pool, sc, br_sb, negc, w_out, i_out, row0, cfg):
    """Group-limited top-8 for one [128 tokens, 64 experts] score tile.

    cfg keys select engine for elementwise work: 'ew' (nc.vector or
    nc.gpsimd), 'gather_split' = how many of the 8 gather ops go to gpsimd.
    """
    P = 128
    ew = nc.gpsimd if cfg.get("ew_gpsimd") else nc.vector

    sb = pool.tile([P, E], F32, tag="sb")
    ew.tensor_add(sb[:], sc[:], br_sb[:])
    sbg = sb[:].rearrange("p (g s) -> p g s", s=GS)

    # group top-2 sum: m1 = group max; m2 = max with m1 removed
    m1 = pool.tile([P, G], F32, tag="m1")
    nc.vector.tensor_reduce(m1[:], sbg, axis=AxX, op=Alu.max)
    eq = pool.tile([P, E], F32, tag="eqg")
    ew.tensor_tensor(
        eq[:].rearrange("p (g s) -> p g s", s=GS), sbg,
        m1[:].unsqueeze(2).to_broadcast([P, G, GS]), op=Alu.is_equal)
    sb2 = pool.tile([P, E], F32, tag="sb2")
    ew.scalar_tensor_tensor(
        out=sb2[:], in0=eq[:], scalar=NEG, in1=sb[:],
        op0=Alu.mult, op1=Alu.add)
    m2 = pool.tile([P, G], F32, tag="m2")
    nc.vector.tensor_reduce(
        m2[:], sb2[:].rearrange("p (g s) -> p g s", s=GS), axis=AxX, op=Alu.max)
    gs_t = pool.tile([P, G], F32, tag="gs")
    ew.tensor_add(gs_t[:], m1[:], m2[:])

    # threshold = 4th largest group score; penalty -1e9 for dropped groups
    g8 = pool.tile([P, 8], F32, tag="g8")
    nc.vector.max(out=g8[:], in_=gs_t[:])
    pen = pool.tile([P, G], F32, tag="pen")
    ew.scalar_tensor_tensor(
        out=pen[:], in0=gs_t[:], scalar=g8[:, 3:4], in1=negc[:],
        op0=Alu.is_lt, op1=Alu.mult)

    mk = pool.tile([P, E], F32, tag="mk")
    ew.tensor_tensor(
        mk[:].rearrange("p (g s) -> p g s", s=GS), sbg,
        pen[:].unsqueeze(2).to_broadcast([P, G, GS]), op=Alu.add)

    # top-8 experts of masked sb (values sorted desc + their indices)
    v8 = pool.tile([P, 8], F32, tag="v8")
    nc.vector.max(out=v8[:], in_=mk[:])
    ix = pool.tile([P, 8], U32, tag="ix")
    nc.vector.max_index(out=ix[:], in_max=v8[:], in_values=mk[:])

    # ordered gather of original scores: (mk == v8[j]) * scores, summed
    gat = pool.tile([P, 8], F32, tag="gat")
    junk = pool.tile([P, E], F32, tag="junk")
    junk2 = pool.tile([P, E], F32, tag="junk2")
    n_gp = cfg.get("gather_gpsimd", 0)
    for j in range(TOPK):
        eng = nc.gpsimd if j < n_gp else nc.vector
        eng.scalar_tensor_tensor(
            out=(junk2 if j < n_gp else junk)[:],
            in0=mk[:], scalar=v8[:, j:j + 1], in1=sc[:],
            op0=Alu.is_equal, op1=Alu.mult, accum_out=gat[:, j:j + 1])

    # normalize * 2.5
    s1 = pool.tile([P, 1], F32, tag="s1")
    nc.vector.tensor_reduce(s1[:], gat[:], axis=AxX, op=Alu.add)
    r1 = pool.tile([P, 1], F32, tag="r1")
    nc.vector.reciprocal(r1[:], s1[:])
    wo = pool.tile([P, 8], F32, tag="wo")
    ew.tensor_scalar(
        out=wo[:], in0=gat[:], scalar1=r1[:, 0:1], scalar2=float(ROUTE_SCALE),
        op0=Alu.mult, op1=Alu.mult)

    nc.sync.dma_start(w_out[row0:row0 + P, :], wo[:])
    nc.sync.dma_start(i_out[row0:row0 + P, :], ix[:].bitcast(I32))


def _body(nc, pools, dram, cfg):
    cpool, xpool, wpool, psA, psB = pools
    xt, w_out, i_out, wt_sb, br_sb, id_sb, negc, br4 = dram
    mode = cfg.get("mode", "full")

    f32mm = cfg.get("f32mm")
    fp8res = cfg.get("fp8res", False)
    CH0 = cfg.get("chunk", CHUNK)
    if cfg.get("ramp") and not fp8res:
        sched = [(0, 256), (256, 256)]
        t = 512
        while t < TPC:
            sched.append((t, CH0))
            t += CH0
    else:
        sched = [(c * CH0, CH0) for c in range(TPC // CH0)]

    if fp8res:
        # xt here is (xh_dram, xd_dram, wd_sb): partition-major token chunks
        xh_dram, xd_dram, wd_sb = xt
        nsp = cfg.get("dma_split", 1)
        for ci, (t0, CH) in enumerate(sched):
            xh_t = xpool.tile([KP, KT * CH], F16, tag="xh")
            xd_t = xpool.tile([KP, KT * CH], F8E4, tag="xd")
            if nsp == 1:
                nc.sync.dma_start(xh_t[:], xh_dram[:, ci, :])
                xde = nc.scalar if cfg.get("xd_act") else nc.sync
                xde.dma_start(xd_t[:], xd_dram[:, ci, :])
            else:
                step = (KT * CH) // nsp
                for q in range(nsp):
                    qs = slice(q * step, (q + 1) * step)
                    nc.sync.dma_start(xh_t[:, qs], xh_dram[:, ci, qs])
                    nc.sync.dma_start(xd_t[:, qs], xd_dram[:, ci, qs])

            if mode == "dma":
                zz = wpool.tile([KP, 1], F32, tag="zz")
                nc.vector.tensor_reduce(zz[:], xh_t[:, 0:8], axis=AxX,
                                        op=Alu.max)
                zz2 = wpool.tile([KP, 1], F32, tag="zz2")
                nc.vector.tensor_reduce(zz2[:], xd_t[:, 0:8].bitcast(mybir.dt.uint8),
                                        axis=AxX, op=Alu.max)
                continue

            xhv = xh_t[:].rearrange("p (k c) -> p k c", k=KT)
            xdv = xd_t[:].rearrange("p (k c) -> p k c", k=KT)
            ps = psA.tile([2 * E, CH], F32, tag="mm")
            NH = max(1, CH // 512)
            for k in range(KT):
                for h in range(NH):
                    hs = slice(h * 512, min((h + 1) * 512, CH))
                    nc.tensor.matmul(ps[:, hs],
                                     wt_sb[:, k * 2 * E:(k + 1) * 2 * E],
                                     xhv[:, k, hs], start=(k == 0 and h == 0),
                                     stop=False)
            for k in range(KT):
                for h in range(NH):
                    hs = slice(h * 512, min((h + 1) * 512, CH))
                    nc.tensor.matmul(ps[0:E, hs], wd_sb[:, k * E:(k + 1) * E],
                                     xdv[:, k, hs], start=False,
                                     stop=(k == KT - 1 and h == NH - 1),
                                     skip_group_check=True)

            ec = nc.vector.tensor_copy if cfg.get("evac") == "dve" \
                else nc.scalar.copy
            NB = CH // 128
            pt = psB.tile([128, NB, E], F32, tag="pt")
            lg = wpool.tile([E, CH], F32, tag="lg")
            ec(lg[:], ps[0:E, :])
            lgB = wpool.tile([E, CH], F32, tag="lgB")
            ec(lgB[:], ps[E:2 * E, :])
            for j in range(NB):
                js = slice(j * 128, (j + 1) * 128)
                nc.tensor.matmul(pt[:, j, :], lg[:, js], id_sb[:],
                                 is_transpose=True, start=True, stop=False)
                nc.tensor.matmul(pt[:, j, :], lgB[:, js], id_sb[:],
                                 is_transpose=True, start=False, stop=True)
            if cfg.get("topk_pair") and mode not in ("mm",):
                # batch two chunks' topk into one NB=2*NB pass (fewer,
                # wider DVE ops)
                if ci % 2 == 0:
                    scW = wpool.tile([128, 2 * NB, E], F32, tag="scW")
                    pair_t0 = t0
                half = ci % 2
                nc.scalar.activation(scW[:, half * NB:(half + 1) * NB, :],
                                     pt[:], Act.Sigmoid)
                if half == 1:
                    _topk_chunk(nc, wpool, scW, br4, w_out, i_out, pair_t0,
                                cfg, 2 * CH)
                continue
            sc = wpool.tile([128, NB, E], F32, tag="sc")
            nc.scalar.activation(sc[:], pt[:], Act.Sigmoid)
            if mode == "mm":
                nc.sync.dma_start(w_out[t0:t0 + 128, :], sc[:, 0, 0:8])
                continue
            _topk_chunk(nc, wpool, sc, br4, w_out, i_out, t0, cfg, CH)
        return

    for t0, CH in sched:
        if f32mm:
            xk = []
            for k in range(KT):
                tl = xpool.tile([KP, CH], F32, tag="xt")
                nc.sync.dma_start(tl[:],
                                  xt[k * KP:(k + 1) * KP, t0:t0 + CH])
                xk.append(tl)
        else:
            xk = []
            for k in range(KT):
                th = xpool.tile([KP, 2, CH], F16, tag="xhl")
                nc.sync.dma_start(th[:],
                                  xt[k * KP:(k + 1) * KP, :, t0:t0 + CH])
                xk.append(th)

        if mode == "dma":
            # consume tiles with a trivial reduce so loads are not dead
            zz = wpool.tile([KP, 1], F32, tag="zz")
            nc.vector.tensor_reduce(zz[:], xk[0][:, 0, 0:8] if not f32mm
                                    else xk[0][:, 0:8], axis=AxX, op=Alu.max)
            continue

        if f32mm:
            ps = psA.tile([E, CH], F32, tag="mm")
            for k in range(KT):
                nc.tensor.matmul(
                    ps[:], wt_sb[:, k * E:(k + 1) * E], xk[k][:],
                    start=(k == 0), stop=(k == KT - 1))
            lg = wpool.tile([E, CH], F32, tag="lg")
            if cfg.get("evac", "act") == "act":
                nc.scalar.copy(lg[:], ps[:])
            else:
                nc.vector.tensor_copy(lg[:], ps[:])
        else:
            # fp32 via fp16 hi/lo split with a packed [wh|wl] stationary:
            # streaming xh then xl through the 128-wide array accumulates
            # psum[0:64]  = wh.xh + wh.xl
            # psum[64:128]= wl.xh + wl.xl
            # so lower+upper = (wh+wl).(xh+xl) = the full-precision product.
            # 2 streams per k-tile instead of 4 (native fp32) or 3 (naive).
            ps = psA.tile([2 * E, CH], F32, tag="mm")
            NH = max(1, CH // 512)
            n = 0
            for k in range(KT):
                wk = wt_sb[:, k * 2 * E:(k + 1) * 2 * E]
                for half in (0, 1):
                    for h in range(NH):
                        hs = slice(h * 512, min((h + 1) * 512, CH))
                        nc.tensor.matmul(ps[:, hs], wk, xk[k][:, half, hs],
                                         start=(n == 0),
                                         stop=(n >= 2 * KT - 1))
                    n += 1
            if cfg.get("acc_tp", True):
                ec = nc.vector.tensor_copy if cfg.get("evac") == "dve" \
                    else nc.scalar.copy
                lg = wpool.tile([E, CH], F32, tag="lg")
                ec(lg[:], ps[0:E, :])
                lgB = wpool.tile([E, CH], F32, tag="lgB")
                ec(lgB[:], ps[E:2 * E, :])
            else:
                tmp = wpool.tile([E, CH], F32, tag="tmphalf")
                nc.scalar.copy(tmp[:], ps[E:2 * E, :])
                lg = wpool.tile([E, CH], F32, tag="lg")
                nc.vector.tensor_add(lg[:], ps[0:E, :], tmp[:])

        if cfg.get("per_tile"):
            for j in range(CH // 128):
                pt = psB.tile([128, E], F32, tag="pt")
                nc.tensor.transpose(pt[:], lg[:, j * 128:(j + 1) * 128],
                                    id_sb[:])
                sc = wpool.tile([128, E], F32, tag="sc")
                nc.scalar.activation(sc[:], pt[:], Act.Sigmoid)
                if mode == "mm":
                    nc.sync.dma_start(
                        w_out[t0 + j * 128:t0 + (j + 1) * 128, :], sc[:, 0:8])
                    continue
                _topk_tile(nc, wpool, sc, br_sb, negc,
                           w_out, i_out, t0 + j * 128, cfg)
            continue

        # blocked layout: token-tiles side by side on the free dim so the
        # elementwise work runs as one wide op per step
        NB = CH // 128
        pt = psB.tile([128, NB, E], F32, tag="pt")
        if not f32mm and cfg.get("acc_tp", True):
            # lg holds [wh-half; wl-half]: accumulate both transposes into
            # the same psum tile -> logits.T without a separate halves-add
            for j in range(NB):
                js = slice(j * 128, (j + 1) * 128)
                nc.tensor.matmul(pt[:, j, :], lg[:, js], id_sb[:],
                                 is_transpose=True, start=True, stop=False)
                nc.tensor.matmul(pt[:, j, :], lgB[:, js], id_sb[:],
                                 is_transpose=True, start=False, stop=True)
        else:
            for j in range(NB):
                nc.tensor.transpose(pt[:, j, :], lg[:, j * 128:(j + 1) * 128],
                                    id_sb[:])
        sc = wpool.tile([128, NB, E], F32, tag="sc")
        nc.scalar.activation(sc[:], pt[:], Act.Sigmoid)
        if mode == "mm":
            nc.sync.dma_start(w_out[t0:t0 + 128, :], sc[:, 0, 0:8])
            continue
        _topk_chunk(nc, wpool, sc, br4, w_out, i_out, t0, cfg, CH)


def _topk_chunk(nc, pool, sc, br4, w_out, i_out, t0, cfg, CH=None):
    """Group-limited top-8 for one [128, NB, 64] blocked score chunk."""
    P = 128
    NB = (CH or cfg.get("chunk", CHUNK)) // 128
    V = nc.vector

    def t4(ap):  # [P, NB, G, GS] view
        return ap.rearrange("p b (g s) -> p b g s", s=GS)

    sb = pool.tile([P, NB, E], F32, tag="sb")
    eng_sb = nc.gpsimd if cfg.get("gp_adds") else V
    eng_sb.tensor_add(sb[:], sc[:], br4[:, 0:NB, :])

    # group top-2 sum: m1 = group max, m2 = max after masking m1 out
    m1 = pool.tile([P, NB, G], F32, tag="m1")
    V.tensor_reduce(m1[:], t4(sb[:]), axis=AxX, op=Alu.max)
    eq = pool.tile([P, NB, E], F32, tag="eqg")
    V.tensor_tensor(t4(eq[:]), t4(sb[:]),
                    m1[:].unsqueeze(3).to_broadcast([P, NB, G, GS]),
                    op=Alu.is_equal)
    sb2 = pool.tile([P, NB, E], F32, tag="sb2")
    V.scalar_tensor_tensor(out=sb2[:], in0=eq[:], scalar=NEG, in1=sb[:],
                           op0=Alu.mult, op1=Alu.add)
    m2 = pool.tile([P, NB, G], F32, tag="m2")
    V.tensor_reduce(m2[:], t4(sb2[:]), axis=AxX, op=Alu.max)
    gs_t = pool.tile([P, NB, G], F32, tag="gs")
    eng_sb.tensor_add(gs_t[:], m1[:], m2[:])

    # per-token group keep-mask: drop groups with rank >= TOPK_G
    if cfg.get("cnt_top4", False):
        # rank[g] = #{g': gs[g'] > gs[g]} via one wide compare + reduce;
        # replaces 4 Max8 calls (high per-op sequencer cost)
        cmp = pool.tile([P, NB, G, G], F32, tag="cmpg")
        V.tensor_tensor(cmp[:],
                        gs_t[:].unsqueeze(2).to_broadcast([P, NB, G, G]),
                        gs_t[:].unsqueeze(3).to_broadcast([P, NB, G, G]),
                        op=Alu.is_gt)
        rank = pool.tile([P, NB, G], F32, tag="rankg")
        V.tensor_reduce(rank[:], cmp[:], axis=AxX, op=Alu.add)
        pen = pool.tile([P, NB, G], F32, tag="pen")
        V.tensor_scalar(out=pen[:], in0=rank[:], scalar1=3.5, scalar2=1.0,
                        op0=Alu.is_gt, op1=Alu.mult)
    else:
        g8 = pool.tile([P, NB, 8], F32, tag="g8")
        for b in range(NB):
            V.max(out=g8[:, b, :], in_=gs_t[:, b, :])
        pen = pool.tile([P, NB, G], F32, tag="pen")
        V.tensor_tensor(pen[:], gs_t[:],
                        g8[:, :, 3:4].to_broadcast([P, NB, G]), op=Alu.is_lt)
    mk = pool.tile([P, NB, E], F32, tag="mk")
    V.scalar_tensor_tensor(
        out=t4(mk[:]),
        in0=pen[:].unsqueeze(3).to_broadcast([P, NB, G, GS]),
        scalar=NEG, in1=t4(sb[:]), op0=Alu.mult, op1=Alu.add)

    # per-token top-8 (sorted values + indices)
    v8 = pool.tile([P, NB, 8], F32, tag="v8")
    ix = pool.tile([P, NB, 8], U32, tag="ix")
    for b in range(NB):
        V.max(out=v8[:, b, :], in_=mk[:, b, :])
        V.max_index(out=ix[:, b, :], in_max=v8[:, b, :], in_values=mk[:, b, :])

    if cfg.get("mode") == "topk1":
        wv = w_out[t0:t0 + NB * 128, :].rearrange("(b p) j -> p b j", p=128)
        iv = i_out[t0:t0 + NB * 128, :].rearrange("(b p) j -> p b j", p=128)
        oeng = nc.scalar if cfg.get("out_dma_act", True) else nc.sync
        oeng.dma_start(wv, v8[:])
        oeng.dma_start(iv, ix[:].bitcast(I32))
        return

    # ordered gather of original scores: (mk == v8[j]) * scores, summed over
    # experts -- fused as 3 wide ops on [P, NB, 8, E] instead of 32 narrow
    # accumulating ops (DVE sequencer-bound otherwise)
    gat = pool.tile([P, NB, 8], F32, tag="gat")
    if cfg.get("gather_split"):
        # split the 8 per-block gather ops between GPSIMD and DVE
        n_gp = cfg["gather_split"]
        junk = pool.tile([P, E], F32, tag="junk")
        junk2 = pool.tile([P, E], F32, tag="junk2")
        for b in range(NB):
            for j in range(TOPK):
                eng = nc.gpsimd if j < n_gp else V
                eng.scalar_tensor_tensor(
                    out=(junk2 if j < n_gp else junk)[:],
                    in0=mk[:, b, :], scalar=v8[:, b, j:j + 1],
                    in1=sc[:, b, :], op0=Alu.is_equal, op1=Alu.mult,
                    accum_out=gat[:, b, j:j + 1])
    elif cfg.get("fused_gather", True):
        GE = nc.gpsimd if cfg.get("gather_gp") else V
        eq4 = pool.tile([P, NB, 8, E], F32, tag="eq4",
                        bufs=cfg.get("gbufs", 2))
        GE.tensor_tensor(
            eq4[:], mk[:].unsqueeze(2).to_broadcast([P, NB, 8, E]),
            v8[:].unsqueeze(3).to_broadcast([P, NB, 8, E]), op=Alu.is_equal)
        prod = pool.tile([P, NB, 8, E],
                         mybir.dt.bfloat16 if cfg.get("prod_bf16") else F32,
                         tag="prod", bufs=cfg.get("gbufs", 2))
        GE.tensor_tensor(
            prod[:], eq4[:], sc[:].unsqueeze(2).to_broadcast([P, NB, 8, E]),
            op=Alu.mult)
        V.tensor_reduce(gat[:], prod[:], axis=AxX, op=Alu.add)
    else:
        junk = pool.tile([P, E], F32, tag="junk")
        for b in range(NB):
            for j in range(TOPK):
                V.scalar_tensor_tensor(
                    out=junk[:], in0=mk[:, b, :], scalar=v8[:, b, j:j + 1],
                    in1=sc[:, b, :], op0=Alu.is_equal, op1=Alu.mult,
                    accum_out=gat[:, b, j:j + 1])

    # weights = 2.5 * gat / sum(gat)
    s1 = pool.tile([P, NB], F32, tag="s1")
    V.tensor_reduce(s1[:], gat[:], axis=AxX, op=Alu.add)
    r1 = pool.tile([P, NB], F32, tag="r1")
    V.reciprocal(r1[:], s1[:])
    wo = pool.tile([P, NB, 8], F32, tag="wo")
    V.scalar_tensor_tensor(
        out=wo[:], in0=gat[:], scalar=float(ROUTE_SCALE),
        in1=r1[:].unsqueeze(2).to_broadcast([P, NB, 8]),
        op0=Alu.mult, op1=Alu.mult)

    wv = w_out[t0:t0 + NB * 128, :].rearrange("(b p) j -> p b j", p=128)
    iv = i_out[t0:t0 + NB * 128, :].rearrange("(b p) j -> p b j", p=128)
    oeng = nc.scalar if cfg.get("out_dma_act", True) else nc.sync
    oeng.dma_start(wv, wo[:])
    oeng.dma_start(iv, ix[:].bitcast(I32))


def _build_nc(n_repeat=1, **cfg):
    import contextlib
    nc = bacc.Bacc(None, target_bir_lowering=False, debug=False)

    fp8res = cfg.get("fp8res", False)
    CH0 = cfg.get("chunk", CHUNK)
    if fp8res:
        nch = TPC // CH0
        xh_d = nc.declare_dram_parameter("xh", [KP, nch, KT * CH0], F16,
                                         isOutput=False)
        xd_d = nc.declare_dram_parameter("xd", [KP, nch, KT * CH0], F8E4,
                                         isOutput=False)
        wt = nc.declare_dram_parameter("whl", [KT * KP, 2 * E], F16,
                                       isOutput=False)
        wd = nc.declare_dram_parameter("wd", [KT * KP, E], F16,
                                       isOutput=False)
        xt = None
    elif cfg.get("f32mm"):
        xt = nc.declare_dram_parameter("xt", [KT * KP, TPC], F32,
                                       isOutput=False)
        wt = nc.declare_dram_parameter("wt", [KT * KP, E], F32, isOutput=False)
    else:
        xt = nc.declare_dram_parameter("xhl", [KT * KP, 2, TPC], F16,
                                       isOutput=False)
        wt = nc.declare_dram_parameter("whl", [KT * KP, 2 * E], F16,
                                       isOutput=False)
    br = nc.declare_dram_parameter("br", [128, E], F32, isOutput=False)
    idn = nc.declare_dram_parameter("idn", [E, E], F32, isOutput=False)
    w_out = nc.declare_dram_parameter("w_out", [TPC, TOPK], F32, isOutput=True)
    i_out = nc.declare_dram_parameter("i_out", [TPC, TOPK], I32, isOutput=True)

    with TileContext(nc) as tc:
        with (
            tc.tile_pool(name="const", bufs=1) as cpool,
            tc.tile_pool(name="xts",
                         bufs=cfg.get("xbufs", 4 if fp8res else 36)) as xpool,
            tc.tile_pool(name="work", bufs=cfg.get("wbufs", 4)) as wpool,
            tc.tile_pool(name="psmm", bufs=cfg.get("psa", 2),
                         space="PSUM") as psA,
            tc.tile_pool(name="pstr", bufs=cfg.get("psb", 4),
                         space="PSUM") as psB,
        ):
            if cfg.get("f32mm"):
                wt_sb = cpool.tile([KP, KT * E], F32)
                nc.sync.dma_start(
                    wt_sb[:].rearrange("p (k e) -> p k e", k=KT),
                    wt[:, :].rearrange("(k p) e -> p k e", p=KP))
            else:
                wt_sb = cpool.tile([KP, KT * 2 * E], F16)
                nc.sync.dma_start(
                    wt_sb[:].rearrange("p (k e) -> p k e", k=KT),
                    wt[:, :].rearrange("(k p) e -> p k e", p=KP))
            if fp8res:
                wd_sb = cpool.tile([KP, KT * E], F16)
                nc.sync.dma_start(
                    wd_sb[:].rearrange("p (k e) -> p k e", k=KT),
                    wd[:, :].rearrange("(k p) e -> p k e", p=KP))
                xt = (xh_d, xd_d, wd_sb)
            br_sb = cpool.tile([128, E], F32)
            nc.sync.dma_start(br_sb[:], br[:, :])
            id_sb = cpool.tile([E, E], F32)
            nc.sync.dma_start(id_sb[:], idn[:, :])
            negc = cpool.tile([128, G], F32)
            nc.vector.memset(negc[:], NEG)
            nbr = (CH0 // 128) * (2 if cfg.get("topk_pair") else 1)
            br4 = cpool.tile([128, nbr, E], F32)
            for b in range(nbr):
                nc.vector.tensor_copy(br4[:, b, :], br_sb[:])

            pools = (cpool, xpool, wpool, psA, psB)
            dram = (xt, w_out, i_out, wt_sb, br_sb, id_sb, negc, br4)
            rep_ctx = tc.For_i(0, n_repeat, 1) if n_repeat > 1 \
                else contextlib.nullcontext()
            with rep_ctx:
                for _ in range(cfg.get("unroll", 1)):
                    _body(nc, pools, dram, cfg)

    nc.compile()
    return nc


def _get_nc():
    if "nc" not in _CACHE:
        _CACHE["nc"] = _build_nc(fp8res=True)
    return _CACHE["nc"]


def _prep_inputs(x, weight, bias, f32mm=False, fp8res=True, chunk=CHUNK):
    import ml_dtypes

    x = np.asarray(x, dtype=np.float32)
    weight = np.asarray(weight, dtype=np.float32)
    bias = np.asarray(bias, dtype=np.float32)
    assert x.shape == (T, DIM) and weight.shape == (E, DIM - 1)

    br = np.tile(bias[None, :], (128, 1)).astype(np.float32)
    idn = np.eye(E, dtype=np.float32)

    wt = np.zeros((KT * KP, E), dtype=np.float32)
    wt[:DIM - 1] = weight.T
    in_maps = []
    if f32mm:
        for c in range(NCORES):
            xtc = np.zeros((KT * KP, TPC), dtype=np.float32)
            xtc[:DIM - 1] = x[c * TPC:(c + 1) * TPC, 1:].T
            in_maps.append({"xt": xtc, "wt": wt, "br": br, "idn": idn})
        return in_maps

    whl = np.empty((KT * KP, 2 * E), dtype=np.float16)
    whl[:, :E] = wt
    whl[:, E:] = wt - whl[:, :E].astype(np.float32)

    if fp8res:
        # fp16 hi + scaled-e4m3 residual, partition-major chunk layout:
        # dram[p, ci, k*CH + c] = feature (k*128+p) of token (ci*CH + c)
        nch = TPC // chunk
        wd = (whl[:, :E].astype(np.float32) * 2.0 ** -SD).astype(np.float16)
        for c in range(NCORES):
            xtc = np.zeros((KT * KP, TPC), dtype=np.float32)
            xtc[:DIM - 1] = x[c * TPC:(c + 1) * TPC, 1:].T
            xh = xtc.astype(np.float16)
            xd32 = (xtc - xh.astype(np.float32)) * float(2.0 ** SD)
            xd = xd32.astype(ml_dtypes.float8_e4m3)
            # [KT*KP, TPC] -> [KT, KP, nch, CH] -> [KP, nch, KT, CH]
            xh_p = np.ascontiguousarray(
                xh.reshape(KT, KP, nch, chunk).transpose(1, 2, 0, 3)
            ).reshape(KP, nch, KT * chunk)
            xd_p = np.ascontiguousarray(
                xd.reshape(KT, KP, nch, chunk).transpose(1, 2, 0, 3)
            ).reshape(KP, nch, KT * chunk)
            in_maps.append({"xh": xh_p, "xd": xd_p, "whl": whl, "wd": wd,
                            "br": br, "idn": idn})
        return in_maps

    for c in range(NCORES):
        xtc = np.zeros((KT * KP, TPC), dtype=np.float32)
        xtc[:DIM - 1] = x[c * TPC:(c + 1) * TPC, 1:].T
        xhl = np.empty((KT * KP, 2, TPC), dtype=np.float16)
        xhl[:, 0, :] = xtc
        xhl[:, 1, :] = xtc - xhl[:, 0, :].astype(np.float32)
        in_maps.append({"xhl": xhl, "whl": whl, "br": br, "idn": idn})
    return in_maps


def kernel(x, weight, bias):
    nc = _get_nc()
    in_maps = _prep_inputs(x, weight, bias)
    out = run_bass_kernel_spmd(nc, in_maps, list(range(NCORES)))
    _CACHE["last_result"] = out
    res = out.results
    weights = np.concatenate([res[c]["w_out"] for c in range(NCORES)], axis=0)
    indices = np.concatenate([res[c]["i_out"] for c in range(NCORES)], axis=0)
    return weights, indices


# ---------------------------------------------------------------------------
# benchmarking helpers (not used by the grader; kernel() above is the entry)
# ---------------------------------------------------------------------------

def _timed_runner(nc, in_maps):
    """Mirror bass2jax.run_bass_via_pjrt's multi-core path, but keep inputs
    resident on device and return a closure that runs + blocks."""
    import jax
    from jax.sharding import Mesh, PartitionSpec, NamedSharding
    from jax.experimental.shard_map import shard_map
    from concourse import bass2jax

    bass2jax.install_neuronx_cc_hook()
    if nc.dbg_addr is not None:
        in_maps = [
            {**m, nc.dbg_addr.name: np.zeros((1, 2), np.uint32)} for m in in_maps
        ]
    partition_name = nc.partition_id_tensor.name if nc.partition_id_tensor else None
    in_names, out_names, out_avals, zero_outs = [], [], [], []
    for alloc in nc.m.functions[0].allocations:
        if not isinstance(alloc, mybir.MemoryLocationSet):
            continue
        name = alloc.memorylocations[0].name
        if alloc.kind == "ExternalInput":
            if name != partition_name:
                in_names.append(name)
        elif alloc.kind == "ExternalOutput":
            shape = tuple(alloc.tensor_shape)
            dtype = mybir.dt.np(alloc.dtype)
            out_names.append(name)
            out_avals.append(jax.core.ShapedArray(shape, dtype))
            zero_outs.append(np.zeros(shape, dtype))
    n_params = len(in_names)
    n_cores = len(in_maps)
    all_in_names = list(in_names) + list(out_names)
    if partition_name is not None:
        all_in_names.append(partition_name)

    def _b(*args):
        operands = list(args)
        if partition_name is not None:
            operands.append(bass2jax.partition_id_tensor())
        outs = bass2jax._bass_exec_p.bind(
            *operands,
            out_avals=tuple(out_avals),
            in_names=tuple(all_in_names),
            out_names=tuple(out_names),
            lowering_input_output_aliases=(),
            sim_require_finite=True,
            sim_require_nnan=True,
            nc=nc,
        )
        return tuple(outs)

    devices = jax.devices()[:n_cores]
    mesh = Mesh(np.asarray(devices), ("core",))
    in_specs = (PartitionSpec("core"),) * (n_params + len(out_names))
    out_specs = (PartitionSpec("core"),) * len(out_names)
    fn = jax.jit(shard_map(_b, mesh=mesh, in_specs=in_specs,
                           out_specs=out_specs, check_rep=False))
    sh = NamedSharding(mesh, PartitionSpec("core"))
    concat_in = [
        jax.device_put(
            np.concatenate([np.asarray(in_maps[c][nm]) for c in range(n_cores)], 0),
            sh)
        for nm in in_names
    ]
    concat_zeros = [
        jax.device_put(np.zeros((n_cores * z.shape[0], *z.shape[1:]), z.dtype), sh)
        for z in zero_outs
    ]

    def run():
        outs = fn(*concat_in, *concat_zeros)
        jax.block_until_ready(outs)
        return outs

    return run


def bench_nc(nc_r, nc_1, in_maps, n_repeat, trials=16):
    import time
    run_r = _timed_runner(nc_r, in_maps)
    run_1 = _timed_runner(nc_1, in_maps)
    run_r(); run_1()
    ts_r, ts_1, deltas = [], [], []
    for _ in range(trials):
        t0 = time.perf_counter(); run_1(); t1 = time.perf_counter()
        run_r(); t2 = time.perf_counter()
        ts_1.append(t1 - t0); ts_r.append(t2 - t1)
        deltas.append((t2 - t1) - (t1 - t0))
    for tag, ts in ((n_repeat, ts_r), (1, ts_1)):
        print(f"    repeat={tag:3d}: min {min(ts)*1e3:8.3f} ms  "
              f"med {sorted(ts)[len(ts)//2]*1e3:8.3f} ms")
    dmin = min(ts_r) - min(ts_1)
    dmed = sorted(deltas)[len(deltas)//2]
    print(f"    delta: min-based {dmin*1e3:7.3f} ms   "
          f"median-paired {dmed*1e3:7.3f} ms")
    return min(dmin, dmed) / (n_repeat - 1) * 1e9  # per-iteration


def bench(x, weight, bias, n_repeat=256, trials=16, **cfg):
    if cfg.get("f32mm") or cfg.get("legacy"):
        cfg["fp8res"] = False
    cfg.setdefault("fp8res", True)
    if cfg["fp8res"]:
        cfg.setdefault("unroll", 4)
    u = cfg.get("unroll", 1)
    n_repeat = n_repeat // u
    in_maps = _prep_inputs(x, weight, bias, f32mm=cfg.get("f32mm", False),
                           fp8res=cfg["fp8res"],
                           chunk=cfg.get("chunk", CHUNK))
    key = tuple(sorted(cfg.items()))
    if ("ncr", key) not in _CACHE:
        _CACHE[("ncr", key)] = _build_nc(n_repeat, **cfg)
        _CACHE[("nc1", key)] = _build_nc(1, **cfg)
    per_iter = bench_nc(_CACHE[("ncr", key)], _CACHE[("nc1", key)],
                        in_maps, n_repeat, trials)
    return per_iter / u

